# revision 38
# baseline (speedup 1.0000x reference)
"""MLA-style multi-head attention kernel for Trainium2, 8-core SPMD.

Problem (hardcoded shapes): B=2, S=2048, E=1024, H=16, DC=256, DR=128, HD=64.
  cq  = gelu(x @ wq + bq)            [B,S,DC]
  q_c = cq @ wqc + bqc               [B,S,896]
  q_r = rope(x @ wqr + bqr)          [B,S,128]
  ckv = gelu(x @ wkv + bkv)          [B,S,DC]
  k_c = ckv @ wkc + bkc;  k_r = rope(x @ wkr + bkr)
  v   = ckv @ wv + bv
  q/k = gelu(concat([*_c, *_r]))  -> 16 heads of 64
  out = causal softmax attention; y = out @ wo + bo

Sharding: core c handles batch b=c//4, head group g=c%4 (heads 4g..4g+3).
Each core computes a partial y (its 256 channels through wo); the host sums
the 4 partials per batch and adds bo.

On-chip layout is channels-on-partitions ("transposed"): per-channel biases
become per-partition scalars fused into ScalarE activations.  Matmuls run in
float32r (TF32-like, 4x faster than fp32 on the PE; set MM_F32R=False for
exact fp32).  Softmax: scores are small (<1.2 scaled) so exp without
max-subtraction; the per-column sum comes from an appended ones-row in the
V operand of the PV matmul; division by the sum uses a K=1 ones-matmul to
broadcast the reciprocal across partitions.
"""
import math
import os
import sys

sys.path.insert(0, "/opt/trn_rl_repo")

import numpy as np

B, S, E = 2, 2048, 1024
H, DC, DR = 16, 256, 128
HD = E // H          # 64
NCORES = 8
HG = 4               # heads per group/core
GC = HG * HD         # 256 group channels
NT = S // 512        # 4 sq tiles of 512
NBLK = S // 128      # 16 sk blocks of 128
EC = E // 128        # 8 k-chunks over E

MM_F32R = os.environ.get("KERNEL_MM_DTYPE", "f32r") == "f32r"
ATT_BF16 = os.environ.get("KERNEL_ATT_DTYPE", "f32r") == "bf16"

_CACHE = {}
LAST_RESULT = [None]


def _build_module():
    import concourse.tile as tile
    from concourse import bacc, mybir

    f32 = mybir.dt.float32
    mmdt = mybir.dt.float32r if MM_F32R else f32
    attdt = mybir.dt.bfloat16 if ATT_BF16 else mmdt
    AF = mybir.ActivationFunctionType
    ALU = mybir.AluOpType

    nc = bacc.Bacc("TRN2", target_bir_lowering=False, debug=False,
                   enable_asserts=True, num_devices=NCORES)

    def inp(name, shape):
        return nc.dram_tensor(name, list(shape), f32, kind="ExternalInput").ap()

    xT = inp("xT", (E, S))
    wq_d = inp("wq", (E, DC))
    wkv_d = inp("wkv", (E, DC))
    wqr_d = inp("wqr", (E, DR))
    wkr_d = inp("wkr", (E, DR))
    wqce_d = inp("wqce", (DC + DR, GC))
    wkce_d = inp("wkce", (DC + DR, GC))
    wv_d = inp("wv", (DC, GC))
    # all per-channel biases packed [128, 12] so they load as one clean DMA:
    # cols 0:2 bq, 2:4 bkv, 4 bqr, 5 bkr, 6:8 bqce, 8:10 bkce, 10:12 bv
    ball_d = inp("ball", (128, 12))
    wo_d = inp("wo", (GC, E))
    cosf_d = inp("cosf", (DR, S))    # cos rows duplicated for both halves
    sins_d = inp("sins", (DR, S))    # sin rows, sign-baked (-sin; +sin)
    dmask_d = inp("dmask", (4, 128, 512))
    ones_d = inp("ones", (128,))
    y_ap = nc.dram_tensor("y", [S, E], f32, kind="ExternalOutput").ap()

    def r(ap):  # reinterpret fp32 dram as matmul dtype
        return ap.bitcast(mmdt) if MM_F32R else ap

    with tile.TileContext(nc) as tc:
        with tc.tile_pool(name="const", bufs=1) as cpool, \
             tc.tile_pool(name="actA", bufs=1) as apool, \
             tc.tile_pool(name="ogrp", bufs=1) as opool, \
             tc.tile_pool(name="rope", bufs=1) as rpool:

            cosf = cpool.tile([DR, S], f32)
            sins = cpool.tile([DR, S], f32)
            # partition dim must be first: store as [128, 4, 512]
            # (dmask/ones1/wo_t DMAs are emitted after phase A so the
            # phase-A weights and x stream win the DMA queues at startup)
            dmask = cpool.tile([128, 4, 512], attdt, name="dmask_t")
            ones1 = cpool.tile([1, 64], mmdt)

            ball = cpool.tile([128, 12], f32)
            bq_t, bkv_t = ball[:, 0:2], ball[:, 2:4]
            bqr_t, bkr_t = ball[:, 4:5], ball[:, 5:6]
            bqce_t, bkce_t, bv_t = ball[:, 6:8], ball[:, 8:10], ball[:, 10:12]

            cq_sb = apool.tile([128, 2, S], mmdt)
            ckv_sb = apool.tile([128, 2, S], mmdt)
            o_grp = opool.tile([128, 2, S], mmdt)
            qrope = rpool.tile([128, S], mmdt)
            krope = rpool.tile([128, S], mmdt)
            wo_t = opool.tile([128, 2, E], mmdt)

            # ---------------- Phase A: E-contraction projections ----------
            with tc.tile_pool(name="wA", bufs=1) as wA, \
                 tc.tile_pool(name="xs", bufs=3) as xs, \
                 tc.tile_pool(name="rawA", bufs=1) as rawA, \
                 tc.tile_pool(name="psA", bufs=1, space="PSUM") as psA:

                wq_t = wA.tile([128, EC, DC], mmdt)
                wkv_t = wA.tile([128, EC, DC], mmdt)
                wqr_t = wA.tile([128, EC, DR], mmdt)
                wkr_t = wA.tile([128, EC, DR], mmdt)
                # per-k-chunk loads so the first matmuls only wait on their
                # own k=0 slices, not the full 3MB of weights
                for k in range(EC):
                    for (wt, wd) in ((wq_t, wq_d), (wkv_t, wkv_d),
                                     (wqr_t, wqr_d), (wkr_t, wkr_d)):
                        nc.gpsimd.dma_start(
                            wt[:, k, :],
                            r(wd.rearrange("(kc p) m -> p kc m", p=128)[:, k, :]))
                nc.gpsimd.dma_start(ball[:], ball_d)
                nc.gpsimd.dma_start(cosf[:], cosf_d)
                nc.gpsimd.dma_start(sins[:], sins_d)

                qr_raw = rawA.tile([128, S], f32)
                kr_raw = rawA.tile([128, S], f32)
                qswp = rawA.tile([128, S], f32)
                kswp = rawA.tile([128, S], f32)

                for n in range(NT):
                    nsl = slice(512 * n, 512 * (n + 1))
                    ps_qr = psA.tile([128, 512], f32, name="ps_qr", bufs=2)
                    ps_kr = psA.tile([128, 512], f32, name="ps_kr", bufs=2)
                    ps_cq = [psA.tile([128, 512], f32, name=f"ps_cq{mc}") for mc in range(2)]
                    ps_ckv = [psA.tile([128, 512], f32, name=f"ps_ckv{mc}") for mc in range(2)]
                    for k in range(EC):
                        xt = xs.tile([128, 512], mmdt, name="xt")
                        nc.sync.dma_start(xt[:], r(xT[128 * k:128 * (k + 1), nsl]))
                        st, sp = (k == 0), (k == EC - 1)
                        nc.tensor.matmul(ps_qr[:], wqr_t[:, k, :], xt[:], start=st, stop=sp)
                        nc.tensor.matmul(ps_kr[:], wkr_t[:, k, :], xt[:], start=st, stop=sp)
                        for mc in range(2):
                            nc.tensor.matmul(ps_cq[mc][:], wq_t[:, k, 128 * mc:128 * (mc + 1)],
                                             xt[:], start=st, stop=sp)
                            nc.tensor.matmul(ps_ckv[mc][:], wkv_t[:, k, 128 * mc:128 * (mc + 1)],
                                             xt[:], start=st, stop=sp)
                    # epilogues: bias via DVE broadcast (qr/kr), Gelu+bias via ACT (cq/ckv)
                    nc.vector.tensor_add(qr_raw[:, nsl], ps_qr[:], bqr_t[:, 0:1].to_broadcast((128, 512)))
                    nc.vector.tensor_add(kr_raw[:, nsl], ps_kr[:], bkr_t[:, 0:1].to_broadcast((128, 512)))
                    for mc in range(2):
                        nc.scalar.activation(cq_sb[:, mc, nsl], ps_cq[mc][:], AF.Gelu,
                                             bias=bq_t[:, mc:mc + 1])
                        nc.scalar.activation(ckv_sb[:, mc, nsl], ps_ckv[mc][:], AF.Gelu,
                                             bias=bkv_t[:, mc:mc + 1])

                    # ------------ Phase B: RoPE (per n-tile) ------------
                    # swapped halves via SBUF->SBUF DMA, then mul/mul/add on DVE
                    for (raw, swp, rp) in ((qr_raw, qswp, qrope), (kr_raw, kswp, krope)):
                        nc.sync.dma_start(swp[0:64, nsl], raw[64:128, nsl])
                        nc.sync.dma_start(swp[64:128, nsl], raw[0:64, nsl])
                        nc.vector.tensor_mul(raw[:, nsl], raw[:, nsl], cosf[:, nsl])
                        nc.vector.tensor_mul(swp[:, nsl], swp[:, nsl], sins[:, nsl])
                        nc.vector.tensor_add(rp[:, nsl], raw[:, nsl], swp[:, nsl])

            # deferred const loads (needed from phase C/D on)
            if ATT_BF16:
                nc.gpsimd.dma_start(dmask[:], dmask_d.rearrange("d p n -> p d n"))
            else:
                nc.gpsimd.dma_start(dmask[:], r(dmask_d.rearrange("d p n -> p d n")))
            nc.gpsimd.dma_start(ones1[:], r(ones_d[None, 0:64]))
            nc.gpsimd.dma_start(wo_t[:], r(wo_d.rearrange("(kc p) m -> p kc m", p=128)))

            # ---------------- Phase C + D + E ------------------------------
            with tc.tile_pool(name="qk", bufs=1) as qkpool, \
                 tc.tile_pool(name="vp", bufs=1) as vpool:
                q_grp = qkpool.tile([128, 2, S], attdt)
                k_grp = qkpool.tile([128, 2, S], attdt)
                v_sb = vpool.tile([128, NBLK, HG, HD + 1], attdt)

                # Phase C: group projections (contraction DC+DR) and V
                with tc.tile_pool(name="wC", bufs=1) as wC, \
                     tc.tile_pool(name="psC", bufs=2, space="PSUM") as psC:
                    wqce_t = wC.tile([128, 3, GC], mmdt)
                    nc.gpsimd.dma_start(wqce_t[:], r(wqce_d.rearrange("(kc p) m -> p kc m", p=128)))
                    wkce_t = wC.tile([128, 3, GC], mmdt)
                    nc.gpsimd.dma_start(wkce_t[:], r(wkce_d.rearrange("(kc p) m -> p kc m", p=128)))
                    wv_t = wC.tile([128, 2, GC], mmdt)
                    nc.gpsimd.dma_start(wv_t[:], r(wv_d.rearrange("(kc p) m -> p kc m", p=128)))
                    ones_col = v_sb[:, :, :, HD:HD + 1].rearrange("p a b c -> p (a b c)")
                    if ATT_BF16:
                        nc.gpsimd.dma_start(ones_col, ones_d[:, None].to_broadcast((128, NBLK * HG)))
                    else:
                        nc.sync.dma_start(ones_col, r(ones_d[:, None].to_broadcast((128, NBLK * HG))))

                    for (grp, wt, bt, rp, csb) in ((q_grp, wqce_t, bqce_t, qrope, cq_sb),
                                                   (k_grp, wkce_t, bkce_t, krope, ckv_sb)):
                        for mc in range(2):
                            for n in range(NT):
                                nsl = slice(512 * n, 512 * (n + 1))
                                ps = psC.tile([128, 512], f32, name="ps_grp")
                                for kc in range(3):
                                    rhs = rp[:, nsl] if kc == 2 else csb[:, kc, nsl]
                                    nc.tensor.matmul(ps[:], wt[:, kc, 128 * mc:128 * (mc + 1)],
                                                     rhs, start=(kc == 0), stop=(kc == 2))
                                nc.scalar.activation(grp[:, mc, nsl], ps[:], AF.Gelu,
                                                     bias=bt[:, mc:mc + 1])
                    for sc in range(NBLK):
                        ps_v = psC.tile([128, GC], f32, name="ps_v")
                        for kc in range(2):
                            nc.tensor.matmul(ps_v[:], ckv_sb[:, kc, 128 * sc:128 * (sc + 1)],
                                             wv_t[:, kc, :], start=(kc == 0), stop=(kc == 1))
                        nc.vector.tensor_copy(v_sb[:, sc, :, 0:HD],
                                              ps_v.rearrange("p (h d) -> p h d", h=HG))

                # Phase D: attention, head-pairs packed into PE row groups;
                # phase E (wo projection) interleaved per sq-tile.
                with tc.tile_pool(name="pTp", bufs=6) as pTp, \
                     tc.tile_pool(name="msc", bufs=3) as msc, \
                     tc.tile_pool(name="ysb", bufs=2) as ysb, \
                     tc.tile_pool(name="psS", bufs=2, space="PSUM") as psS, \
                     tc.tile_pool(name="psO", bufs=1, space="PSUM") as psO, \
                     tc.tile_pool(name="psB", bufs=1, space="PSUM") as psB, \
                     tc.tile_pool(name="psE", bufs=1, space="PSUM") as psE:
                    # wo-projection groups of tile t-1 are spread through tile
                    # t's attention loop: the 128x128 matmuls keep the PE HAM
                    # clock warm (half-array QK/PV alone lets it drop) and
                    # overlap the output DMA with attention.
                    pending_e = []

                    def emit_e(sc, n2):
                        ps_y = psE.tile([128, 512], f32, name="ps_y")
                        for kc in range(2):
                            nc.tensor.matmul(ps_y[:], o_grp[:, kc, 128 * sc:128 * (sc + 1)],
                                             wo_t[:, kc, 512 * n2:512 * (n2 + 1)],
                                             start=(kc == 0), stop=(kc == 1))
                        y_sb = ysb.tile([128, 512], f32, name="y_sb")
                        nc.vector.tensor_copy(y_sb[:], ps_y[:])
                        nc.sync.dma_start(
                            y_ap[128 * sc:128 * (sc + 1), 512 * n2:512 * (n2 + 1)],
                            y_sb[:])

                    for t in range(NT):
                        tsl = slice(512 * t, 512 * (t + 1))
                        nblk = 4 * t + 4
                        for hp in range(2):
                            ps_oa = psO.tile([HD + 1, 512], f32, name="ps_oa")
                            ps_ob = psO.tile([HD + 1, 512], f32, name="ps_ob")
                            for j in range(nblk):
                                jsl = slice(128 * j, 128 * (j + 1))
                                # diagonal block d = j-4t only has visible
                                # columns >= 128*d: run the whole chain on the
                                # narrower column slice
                                off = max(0, 128 * (j - 4 * t))
                                csl = slice(off, 512)
                                qsl = slice(512 * t + off, 512 * (t + 1))
                                ps_a = psS.tile([128, 512], f32, name="ps_a")
                                ps_b = psS.tile([128, 512], f32, name="ps_b")
                                # two heads packed into PE rows 0-63 / 64-127
                                nc.tensor.matmul(ps_a[:, csl], k_grp[0:64, hp, jsl],
                                                 q_grp[0:64, hp, qsl], start=True, stop=True)
                                nc.tensor.matmul(ps_b[:, csl], k_grp[64:128, hp, jsl],
                                                 q_grp[64:128, hp, qsl], start=True, stop=True)
                                pT_a = pTp.tile([128, 512], attdt, name="pT_a")
                                nc.scalar.activation(pT_a[:, csl], ps_a[:, csl], AF.Exp,
                                                     scale=1.0 / math.sqrt(HD))
                                pT_b = pTp.tile([128, 512], attdt, name="pT_b")
                                nc.scalar.activation(pT_b[:, csl], ps_b[:, csl], AF.Exp,
                                                     scale=1.0 / math.sqrt(HD))
                                if j >= 4 * t:
                                    dm = dmask[:, j - 4 * t, csl]
                                    # gpsimd offloads the f32r masks; bf16 DVE is fast
                                    meng = nc.vector if ATT_BF16 else nc.gpsimd
                                    meng.tensor_mul(pT_a[:, csl], pT_a[:, csl], dm)
                                    meng.tensor_mul(pT_b[:, csl], pT_b[:, csl], dm)
                                nc.tensor.matmul(ps_oa[:, csl], v_sb[:, j, 2 * hp, :], pT_a[:, csl],
                                                 start=(j == 0), stop=(j == nblk - 1))
                                nc.tensor.matmul(ps_ob[:, csl], v_sb[:, j, 2 * hp + 1, :], pT_b[:, csl],
                                                 start=(j == 0), stop=(j == nblk - 1))
                                if pending_e and j % 3 == 1:
                                    emit_e(*pending_e.pop(0))
                            for (half, ps_o) in ((0, ps_oa), (1, ps_ob)):
                                r0 = 64 * half
                                recip = msc.tile([1, 512], mmdt, name="recip")
                                with nc.allow_low_precision(reason="softmax recip feeds broadcast"):
                                    nc.vector.reciprocal(recip[:], ps_o[HD:HD + 1, :])
                                ps_rb = psB.tile([64, 512], f32, name="ps_rb")
                                nc.tensor.matmul(ps_rb[:], ones1[:], recip[:], start=True, stop=True)
                                rb_sb = msc.tile([64, 512], f32, name="rb_sb")
                                nc.vector.tensor_copy(rb_sb[:], ps_rb[:])
                                og = o_grp[r0:r0 + 64, hp, tsl]
                                nc.vector.tensor_mul(og, ps_o[0:HD, :], rb_sb[:])
                                nc.vector.tensor_add(og, og, bv_t[r0:r0 + 64, hp:hp + 1]
                                                     .to_broadcast((64, 512)))
                        # queue this sq-tile's wo-projection groups; they are
                        # emitted inside tile t+1's attention loop
                        pending_e += [(sc, n2) for sc in range(4 * t, 4 * t + 4)
                                      for n2 in range(2)]
                    for g in pending_e:
                        emit_e(*g)

    nc.compile()
    from concourse.bass_interp import get_hw_module
    nc.m = get_hw_module(nc.m)
    return nc


def _host_consts():
    inv_freq = 1.0 / (10000.0 ** (np.arange(0, DR, 2, dtype=np.float64) / DR))
    t = np.arange(S, dtype=np.float64)
    fr = t[:, None] * inv_freq[None, :]              # [S, 64]
    cosT = np.cos(fr).T.astype(np.float32)           # [64, S]
    sinT = np.sin(fr).T.astype(np.float32)
    cosf = np.concatenate([cosT, cosT], axis=0)      # [128, S]
    sins = np.concatenate([-sinT, sinT], axis=0)
    j = np.arange(128)[:, None]
    i = np.arange(512)[None, :]
    dmask = np.stack([(j + 128 * d <= i) for d in range(4)]).astype(np.float32)
    return cosf, sins, dmask


def kernel(x, wq, bq, wqc, bqc, wqr, bqr, wkv, bkv, wkr, bkr, wkc, bkc, wv, bv, wo, bo):
    trace = os.environ.get("KERNEL_TRACE") == "1"
    if trace:
        import types
        import antenv  # noqa
        if "antenv.axon_hooks" not in sys.modules:
            _hb = [None]
            _m = types.ModuleType("antenv.axon_hooks")
            _m.set_axon_ntff_profile_hook = lambda h: _hb.__setitem__(0, h)
            _m.get_axon_ntff_profile_hook = lambda: _hb[0]
            sys.modules["antenv.axon_hooks"] = _m
            from trn_agent_boot.trn_boot import _ntff_profile_via_ctypes
            _m.set_axon_ntff_profile_hook(_ntff_profile_via_ctypes("/opt/axon/libaxon_pjrt.so"))
        import concourse.bass_utils as bass_utils
        bass_utils.upload_artifacts = lambda tmpdir: tmpdir

    if "nc" not in _CACHE:
        _CACHE["nc"] = _build_module()
    nc = _CACHE["nc"]

    x = np.ascontiguousarray(np.asarray(x, dtype=np.float32))
    cosf, sins, dmask = _host_consts()
    ones = np.ones(128, np.float32)
    f32c = lambda a: np.ascontiguousarray(np.asarray(a, dtype=np.float32))

    in_maps = []
    for c in range(NCORES):
        b, g = c // 4, c % 4
        gsl = slice(GC * g, GC * (g + 1))
        wqce = np.zeros((DC + DR, GC), np.float32)
        bqce = np.zeros(GC, np.float32)
        wkce = np.zeros((DC + DR, GC), np.float32)
        bkce = np.zeros(GC, np.float32)
        if g < 3:
            wqce[:DC] = wqc[:, gsl]
            bqce[:] = bqc[gsl]
            wkce[:DC] = wkc[:, gsl]
            bkce[:] = bkc[gsl]
        else:
            wqce[:DC, :DR] = wqc[:, 768:896]
            bqce[:DR] = bqc[768:896]
            wqce[DC:, DR:] = np.eye(DR, dtype=np.float32)
            wkce[:DC, :DR] = wkc[:, 768:896]
            bkce[:DR] = bkc[768:896]
            wkce[DC:, DR:] = np.eye(DR, dtype=np.float32)
        ball = np.zeros((128, 12), np.float32)
        ball[:, 0:2] = f32c(bq).reshape(2, 128).T
        ball[:, 2:4] = f32c(bkv).reshape(2, 128).T
        ball[:, 4] = f32c(bqr)
        ball[:, 5] = f32c(bkr)
        ball[:, 6:8] = bqce.reshape(2, 128).T
        ball[:, 8:10] = bkce.reshape(2, 128).T
        ball[:, 10:12] = f32c(bv[gsl]).reshape(2, 128).T
        in_maps.append(dict(
            xT=np.ascontiguousarray(x[b].T),
            wq=f32c(wq), wkv=f32c(wkv),
            wqr=f32c(wqr), wkr=f32c(wkr),
            wqce=wqce, wkce=wkce,
            wv=f32c(wv[:, gsl]),
            wo=f32c(wo[gsl, :]),
            ball=ball,
            cosf=cosf, sins=sins, dmask=dmask, ones=ones,
        ))

    from concourse.bass_utils import run_bass_kernel_spmd
    res = run_bass_kernel_spmd(nc, in_maps, core_ids=list(range(NCORES)), trace=trace)
    LAST_RESULT[0] = res

    y = np.empty((B, S, E), np.float32)
    for b in range(B):
        acc = res.results[4 * b]["y"].astype(np.float32).copy()
        for g in range(1, 4):
            acc += res.results[4 * b + g]["y"]
        y[b] = acc + np.asarray(bo, dtype=np.float32)[None, :]
    return y


# revision 39
# speedup vs baseline: 1.0421x; 1.0421x over previous
"""MLA-style multi-head attention kernel for Trainium2, 8-core SPMD.

Problem (hardcoded shapes): B=2, S=2048, E=1024, H=16, DC=256, DR=128, HD=64.
  cq  = gelu(x @ wq + bq)            [B,S,DC]
  q_c = cq @ wqc + bqc               [B,S,896]
  q_r = rope(x @ wqr + bqr)          [B,S,128]
  ckv = gelu(x @ wkv + bkv)          [B,S,DC]
  k_c = ckv @ wkc + bkc;  k_r = rope(x @ wkr + bkr)
  v   = ckv @ wv + bv
  q/k = gelu(concat([*_c, *_r]))  -> 16 heads of 64
  out = causal softmax attention; y = out @ wo + bo

Sharding: core c handles batch b=c//4, head group g=c%4 (heads 4g..4g+3).
Each core computes a partial y (its 256 channels through wo); the host sums
the 4 partials per batch and adds bo.

On-chip layout is channels-on-partitions ("transposed"): per-channel biases
become per-partition scalars fused into ScalarE activations.  Matmuls run in
float32r (TF32-like, 4x faster than fp32 on the PE; set MM_F32R=False for
exact fp32).  Softmax: scores are small (<1.2 scaled) so exp without
max-subtraction; the per-column sum comes from an appended ones-row in the
V operand of the PV matmul; division by the sum uses a K=1 ones-matmul to
broadcast the reciprocal across partitions.
"""
import math
import os
import sys

sys.path.insert(0, "/opt/trn_rl_repo")

import numpy as np

B, S, E = 2, 2048, 1024
H, DC, DR = 16, 256, 128
HD = E // H          # 64
NCORES = 8
HG = 4               # heads per group/core
GC = HG * HD         # 256 group channels
NT = S // 512        # 4 sq tiles of 512
NBLK = S // 128      # 16 sk blocks of 128
EC = E // 128        # 8 k-chunks over E

MM_F32R = os.environ.get("KERNEL_MM_DTYPE", "f32r") == "f32r"
ATT_BF16 = os.environ.get("KERNEL_ATT_DTYPE", "f32r") == "bf16"

_CACHE = {}
LAST_RESULT = [None]


def _build_module():
    import concourse.tile as tile
    from concourse import bacc, mybir

    f32 = mybir.dt.float32
    mmdt = mybir.dt.float32r if MM_F32R else f32
    attdt = mybir.dt.bfloat16 if ATT_BF16 else mmdt
    AF = mybir.ActivationFunctionType
    ALU = mybir.AluOpType

    nc = bacc.Bacc("TRN2", target_bir_lowering=False, debug=False,
                   enable_asserts=True, num_devices=NCORES)

    def inp(name, shape):
        return nc.dram_tensor(name, list(shape), f32, kind="ExternalInput").ap()

    xT = inp("xT", (E, S))
    wq_d = inp("wq", (E, DC))
    wkv_d = inp("wkv", (E, DC))
    wqr_d = inp("wqr", (E, DR))
    wkr_d = inp("wkr", (E, DR))
    wqce_d = inp("wqce", (DC + DR, GC))
    wkce_d = inp("wkce", (DC + DR, GC))
    wv_d = inp("wv", (DC, GC))
    # all per-channel biases packed [128, 12] so they load as one clean DMA:
    # cols 0:2 bq, 2:4 bkv, 4 bqr, 5 bkr, 6:8 bqce, 8:10 bkce, 10:12 bv
    ball_d = inp("ball", (128, 12))
    wo_d = inp("wo", (GC, E))
    cosf_d = inp("cosf", (DR, S))    # cos rows duplicated for both halves
    sins_d = inp("sins", (DR, S))    # sin rows, sign-baked (-sin; +sin)
    dmask_d = inp("dmask", (4, 128, 512))
    ones_d = inp("ones", (128,))
    y_ap = nc.dram_tensor("y", [S, E], f32, kind="ExternalOutput").ap()

    def r(ap):  # reinterpret fp32 dram as matmul dtype
        return ap.bitcast(mmdt) if MM_F32R else ap

    with tile.TileContext(nc) as tc:
        with tc.tile_pool(name="const", bufs=1) as cpool, \
             tc.tile_pool(name="actA", bufs=1) as apool, \
             tc.tile_pool(name="ogrp", bufs=1) as opool, \
             tc.tile_pool(name="rope", bufs=1) as rpool, \
             tc.tile_pool(name="psP", bufs=1, space="PSUM") as psP:

            cosf = cpool.tile([DR, S], f32)
            sins = cpool.tile([DR, S], f32)
            # partition dim must be first: store as [128, 4, 512]
            # (dmask/ones1/wo_t DMAs are emitted after phase A so the
            # phase-A weights and x stream win the DMA queues at startup)
            dmask = cpool.tile([128, 4, 512], attdt, name="dmask_t")
            ones1 = cpool.tile([1, 64], mmdt)

            ball = cpool.tile([128, 12], f32)
            bq_t, bkv_t = ball[:, 0:2], ball[:, 2:4]
            bqr_t, bkr_t = ball[:, 4:5], ball[:, 5:6]
            bqce_t, bkce_t, bv_t = ball[:, 6:8], ball[:, 8:10], ball[:, 10:12]

            cq_sb = apool.tile([128, 2, S], mmdt)
            ckv_sb = apool.tile([128, 2, S], mmdt)
            o_grp = opool.tile([128, 2, S], mmdt)
            qrope = rpool.tile([128, S], mmdt)
            krope = rpool.tile([128, S], mmdt)
            wo_t = opool.tile([128, 2, E], mmdt)

            # ---------------- Phase A: E-contraction projections ----------
            with tc.tile_pool(name="wA", bufs=1) as wA, \
                 tc.tile_pool(name="xs", bufs=3) as xs, \
                 tc.tile_pool(name="rawA", bufs=1) as rawA:

                wq_t = wA.tile([128, EC, DC], mmdt)
                wkv_t = wA.tile([128, EC, DC], mmdt)
                wqr_t = wA.tile([128, EC, DR], mmdt)
                wkr_t = wA.tile([128, EC, DR], mmdt)
                # per-k-chunk loads so the first matmuls only wait on their
                # own k=0 slices, not the full 3MB of weights
                for k in range(EC):
                    for (wt, wd) in ((wq_t, wq_d), (wkv_t, wkv_d),
                                     (wqr_t, wqr_d), (wkr_t, wkr_d)):
                        nc.gpsimd.dma_start(
                            wt[:, k, :],
                            r(wd.rearrange("(kc p) m -> p kc m", p=128)[:, k, :]))
                nc.gpsimd.dma_start(ball[:], ball_d)
                nc.gpsimd.dma_start(cosf[:], cosf_d)
                nc.gpsimd.dma_start(sins[:], sins_d)

                qr_raw = rawA.tile([128, S], f32)
                kr_raw = rawA.tile([128, S], f32)
                qswp = rawA.tile([128, S], f32)
                kswp = rawA.tile([128, S], f32)

                for n in range(NT):
                    nsl = slice(512 * n, 512 * (n + 1))
                    ps_qr = psP.tile([128, 512], f32, name="ps_qr", tag="bankA", bufs=2)
                    ps_kr = psP.tile([128, 512], f32, name="ps_kr", tag="bankB", bufs=2)
                    ps_cq = [psP.tile([128, 512], f32, name=f"ps_cq{mc}", tag=f"bankC{mc}", bufs=1) for mc in range(2)]
                    ps_ckv = [psP.tile([128, 512], f32, name=f"ps_ckv{mc}", tag=f"bankE{mc}", bufs=1) for mc in range(2)]
                    for k in range(EC):
                        xt = xs.tile([128, 512], mmdt, name="xt")
                        nc.sync.dma_start(xt[:], r(xT[128 * k:128 * (k + 1), nsl]))
                        st, sp = (k == 0), (k == EC - 1)
                        nc.tensor.matmul(ps_qr[:], wqr_t[:, k, :], xt[:], start=st, stop=sp)
                        nc.tensor.matmul(ps_kr[:], wkr_t[:, k, :], xt[:], start=st, stop=sp)
                        for mc in range(2):
                            nc.tensor.matmul(ps_cq[mc][:], wq_t[:, k, 128 * mc:128 * (mc + 1)],
                                             xt[:], start=st, stop=sp)
                            nc.tensor.matmul(ps_ckv[mc][:], wkv_t[:, k, 128 * mc:128 * (mc + 1)],
                                             xt[:], start=st, stop=sp)
                    # epilogues: bias via DVE broadcast (qr/kr), Gelu+bias via ACT (cq/ckv)
                    nc.vector.tensor_add(qr_raw[:, nsl], ps_qr[:], bqr_t[:, 0:1].to_broadcast((128, 512)))
                    nc.vector.tensor_add(kr_raw[:, nsl], ps_kr[:], bkr_t[:, 0:1].to_broadcast((128, 512)))
                    for mc in range(2):
                        nc.scalar.activation(cq_sb[:, mc, nsl], ps_cq[mc][:], AF.Gelu,
                                             bias=bq_t[:, mc:mc + 1])
                        nc.scalar.activation(ckv_sb[:, mc, nsl], ps_ckv[mc][:], AF.Gelu,
                                             bias=bkv_t[:, mc:mc + 1])

                    # ------------ Phase B: RoPE (per n-tile) ------------
                    # swapped halves via SBUF->SBUF DMA, then mul/mul/add on DVE
                    for (raw, swp, rp) in ((qr_raw, qswp, qrope), (kr_raw, kswp, krope)):
                        nc.sync.dma_start(swp[0:64, nsl], raw[64:128, nsl])
                        nc.sync.dma_start(swp[64:128, nsl], raw[0:64, nsl])
                        nc.vector.tensor_mul(raw[:, nsl], raw[:, nsl], cosf[:, nsl])
                        nc.vector.tensor_mul(swp[:, nsl], swp[:, nsl], sins[:, nsl])
                        nc.vector.tensor_add(rp[:, nsl], raw[:, nsl], swp[:, nsl])

            # deferred const loads (needed from phase C/D on)
            if ATT_BF16:
                nc.gpsimd.dma_start(dmask[:], dmask_d.rearrange("d p n -> p d n"))
            else:
                nc.gpsimd.dma_start(dmask[:], r(dmask_d.rearrange("d p n -> p d n")))
            nc.gpsimd.dma_start(ones1[:], r(ones_d[None, 0:64]))
            nc.gpsimd.dma_start(wo_t[:], r(wo_d.rearrange("(kc p) m -> p kc m", p=128)))

            # ---------------- Phase C + D + E ------------------------------
            with tc.tile_pool(name="qk", bufs=1) as qkpool, \
                 tc.tile_pool(name="vp", bufs=1) as vpool:
                q_grp = qkpool.tile([128, 2, S], attdt)
                k_grp = qkpool.tile([128, 2, S], attdt)
                v_sb = vpool.tile([128, NBLK, HG, HD + 1], attdt)

                # Phase C: group projections (contraction DC+DR) and V
                with tc.tile_pool(name="wC", bufs=1) as wC:
                    wqce_t = wC.tile([128, 3, GC], mmdt)
                    nc.gpsimd.dma_start(wqce_t[:], r(wqce_d.rearrange("(kc p) m -> p kc m", p=128)))
                    wkce_t = wC.tile([128, 3, GC], mmdt)
                    nc.gpsimd.dma_start(wkce_t[:], r(wkce_d.rearrange("(kc p) m -> p kc m", p=128)))
                    wv_t = wC.tile([128, 2, GC], mmdt)
                    nc.gpsimd.dma_start(wv_t[:], r(wv_d.rearrange("(kc p) m -> p kc m", p=128)))
                    ones_col = v_sb[:, :, :, HD:HD + 1].rearrange("p a b c -> p (a b c)")
                    if ATT_BF16:
                        nc.gpsimd.dma_start(ones_col, ones_d[:, None].to_broadcast((128, NBLK * HG)))
                    else:
                        nc.sync.dma_start(ones_col, r(ones_d[:, None].to_broadcast((128, NBLK * HG))))

                    for (grp, wt, bt, rp, csb) in ((q_grp, wqce_t, bqce_t, qrope, cq_sb),
                                                   (k_grp, wkce_t, bkce_t, krope, ckv_sb)):
                        for mc in range(2):
                            for n in range(NT):
                                nsl = slice(512 * n, 512 * (n + 1))
                                ps = psP.tile([128, 512], f32, name="ps_grp", tag="bankA", bufs=2)
                                for kc in range(3):
                                    rhs = rp[:, nsl] if kc == 2 else csb[:, kc, nsl]
                                    nc.tensor.matmul(ps[:], wt[:, kc, 128 * mc:128 * (mc + 1)],
                                                     rhs, start=(kc == 0), stop=(kc == 2))
                                nc.scalar.activation(grp[:, mc, nsl], ps[:], AF.Gelu,
                                                     bias=bt[:, mc:mc + 1])
                    for sc in range(NBLK):
                        ps_v = psP.tile([128, GC], f32, name="ps_v", tag="bankB", bufs=2)
                        for kc in range(2):
                            nc.tensor.matmul(ps_v[:], ckv_sb[:, kc, 128 * sc:128 * (sc + 1)],
                                             wv_t[:, kc, :], start=(kc == 0), stop=(kc == 1))
                        nc.vector.tensor_copy(v_sb[:, sc, :, 0:HD],
                                              ps_v.rearrange("p (h d) -> p h d", h=HG))

                # Phase D: attention, head-pairs packed into PE row groups;
                # phase E (wo projection) interleaved per sq-tile.
                with tc.tile_pool(name="pTp", bufs=6) as pTp, \
                     tc.tile_pool(name="msc", bufs=3) as msc, \
                     tc.tile_pool(name="ysb", bufs=2) as ysb:
                    # wo-projection groups of tile t-1 are spread through tile
                    # t's attention loop: the 128x128 matmuls keep the PE HAM
                    # clock warm (half-array QK/PV alone lets it drop) and
                    # overlap the output DMA with attention.
                    pending_e = []

                    def emit_e(sc, n2):
                        ps_y = psP.tile([128, 512], f32, name="ps_y", tag="bankE1", bufs=1)
                        for kc in range(2):
                            nc.tensor.matmul(ps_y[:], o_grp[:, kc, 128 * sc:128 * (sc + 1)],
                                             wo_t[:, kc, 512 * n2:512 * (n2 + 1)],
                                             start=(kc == 0), stop=(kc == 1))
                        y_sb = ysb.tile([128, 512], f32, name="y_sb")
                        nc.vector.tensor_copy(y_sb[:], ps_y[:])
                        nc.sync.dma_start(
                            y_ap[128 * sc:128 * (sc + 1), 512 * n2:512 * (n2 + 1)],
                            y_sb[:])

                    for t in range(NT):
                        tsl = slice(512 * t, 512 * (t + 1))
                        nblk = 4 * t + 4
                        for hp in range(2):
                            ps_oa = psP.tile([HD + 1, 512], f32, name="ps_oa", tag="bankC0", bufs=1)
                            ps_ob = psP.tile([HD + 1, 512], f32, name="ps_ob", tag="bankC1", bufs=1)
                            for j in range(nblk):
                                jsl = slice(128 * j, 128 * (j + 1))
                                # diagonal block d = j-4t only has visible
                                # columns >= 128*d: run the whole chain on the
                                # narrower column slice
                                off = max(0, 128 * (j - 4 * t))
                                csl = slice(off, 512)
                                qsl = slice(512 * t + off, 512 * (t + 1))
                                ps_a = psP.tile([128, 512], f32, name="ps_a", tag="bankA", bufs=2)
                                ps_b = psP.tile([128, 512], f32, name="ps_b", tag="bankB", bufs=2)
                                # two heads packed into PE rows 0-63 / 64-127
                                nc.tensor.matmul(ps_a[:, csl], k_grp[0:64, hp, jsl],
                                                 q_grp[0:64, hp, qsl], start=True, stop=True)
                                nc.tensor.matmul(ps_b[:, csl], k_grp[64:128, hp, jsl],
                                                 q_grp[64:128, hp, qsl], start=True, stop=True)
                                pT_a = pTp.tile([128, 512], attdt, name="pT_a")
                                nc.scalar.activation(pT_a[:, csl], ps_a[:, csl], AF.Exp,
                                                     scale=1.0 / math.sqrt(HD))
                                pT_b = pTp.tile([128, 512], attdt, name="pT_b")
                                nc.scalar.activation(pT_b[:, csl], ps_b[:, csl], AF.Exp,
                                                     scale=1.0 / math.sqrt(HD))
                                if j >= 4 * t:
                                    dm = dmask[:, j - 4 * t, csl]
                                    # gpsimd offloads the f32r masks; bf16 DVE is fast
                                    meng = nc.vector if ATT_BF16 else nc.gpsimd
                                    meng.tensor_mul(pT_a[:, csl], pT_a[:, csl], dm)
                                    meng.tensor_mul(pT_b[:, csl], pT_b[:, csl], dm)
                                nc.tensor.matmul(ps_oa[:, csl], v_sb[:, j, 2 * hp, :], pT_a[:, csl],
                                                 start=(j == 0), stop=(j == nblk - 1))
                                nc.tensor.matmul(ps_ob[:, csl], v_sb[:, j, 2 * hp + 1, :], pT_b[:, csl],
                                                 start=(j == 0), stop=(j == nblk - 1))
                                if pending_e and j % 3 == 1:
                                    emit_e(*pending_e.pop(0))
                            for (half, ps_o) in ((0, ps_oa), (1, ps_ob)):
                                r0 = 64 * half
                                recip = msc.tile([1, 512], mmdt, name="recip")
                                with nc.allow_low_precision(reason="softmax recip feeds broadcast"):
                                    nc.vector.reciprocal(recip[:], ps_o[HD:HD + 1, :])
                                ps_rb = psP.tile([64, 512], f32, name="ps_rb", tag="bankE0", bufs=1)
                                nc.tensor.matmul(ps_rb[:], ones1[:], recip[:], start=True, stop=True)
                                rb_sb = msc.tile([64, 512], f32, name="rb_sb")
                                nc.vector.tensor_copy(rb_sb[:], ps_rb[:])
                                og = o_grp[r0:r0 + 64, hp, tsl]
                                nc.vector.tensor_mul(og, ps_o[0:HD, :], rb_sb[:])
                                nc.vector.tensor_add(og, og, bv_t[r0:r0 + 64, hp:hp + 1]
                                                     .to_broadcast((64, 512)))
                        # queue this sq-tile's wo-projection groups; they are
                        # emitted inside tile t+1's attention loop
                        pending_e += [(sc, n2) for sc in range(4 * t, 4 * t + 4)
                                      for n2 in range(2)]
                    for g in pending_e:
                        emit_e(*g)

    nc.compile()
    from concourse.bass_interp import get_hw_module
    nc.m = get_hw_module(nc.m)
    return nc


def _host_consts():
    inv_freq = 1.0 / (10000.0 ** (np.arange(0, DR, 2, dtype=np.float64) / DR))
    t = np.arange(S, dtype=np.float64)
    fr = t[:, None] * inv_freq[None, :]              # [S, 64]
    cosT = np.cos(fr).T.astype(np.float32)           # [64, S]
    sinT = np.sin(fr).T.astype(np.float32)
    cosf = np.concatenate([cosT, cosT], axis=0)      # [128, S]
    sins = np.concatenate([-sinT, sinT], axis=0)
    j = np.arange(128)[:, None]
    i = np.arange(512)[None, :]
    dmask = np.stack([(j + 128 * d <= i) for d in range(4)]).astype(np.float32)
    return cosf, sins, dmask


def kernel(x, wq, bq, wqc, bqc, wqr, bqr, wkv, bkv, wkr, bkr, wkc, bkc, wv, bv, wo, bo):
    trace = os.environ.get("KERNEL_TRACE") == "1"
    if trace:
        import types
        import antenv  # noqa
        if "antenv.axon_hooks" not in sys.modules:
            _hb = [None]
            _m = types.ModuleType("antenv.axon_hooks")
            _m.set_axon_ntff_profile_hook = lambda h: _hb.__setitem__(0, h)
            _m.get_axon_ntff_profile_hook = lambda: _hb[0]
            sys.modules["antenv.axon_hooks"] = _m
            from trn_agent_boot.trn_boot import _ntff_profile_via_ctypes
            _m.set_axon_ntff_profile_hook(_ntff_profile_via_ctypes("/opt/axon/libaxon_pjrt.so"))
        import concourse.bass_utils as bass_utils
        bass_utils.upload_artifacts = lambda tmpdir: tmpdir

    if "nc" not in _CACHE:
        _CACHE["nc"] = _build_module()
    nc = _CACHE["nc"]

    x = np.ascontiguousarray(np.asarray(x, dtype=np.float32))
    cosf, sins, dmask = _host_consts()
    ones = np.ones(128, np.float32)
    f32c = lambda a: np.ascontiguousarray(np.asarray(a, dtype=np.float32))

    in_maps = []
    for c in range(NCORES):
        b, g = c // 4, c % 4
        gsl = slice(GC * g, GC * (g + 1))
        wqce = np.zeros((DC + DR, GC), np.float32)
        bqce = np.zeros(GC, np.float32)
        wkce = np.zeros((DC + DR, GC), np.float32)
        bkce = np.zeros(GC, np.float32)
        if g < 3:
            wqce[:DC] = wqc[:, gsl]
            bqce[:] = bqc[gsl]
            wkce[:DC] = wkc[:, gsl]
            bkce[:] = bkc[gsl]
        else:
            wqce[:DC, :DR] = wqc[:, 768:896]
            bqce[:DR] = bqc[768:896]
            wqce[DC:, DR:] = np.eye(DR, dtype=np.float32)
            wkce[:DC, :DR] = wkc[:, 768:896]
            bkce[:DR] = bkc[768:896]
            wkce[DC:, DR:] = np.eye(DR, dtype=np.float32)
        ball = np.zeros((128, 12), np.float32)
        ball[:, 0:2] = f32c(bq).reshape(2, 128).T
        ball[:, 2:4] = f32c(bkv).reshape(2, 128).T
        ball[:, 4] = f32c(bqr)
        ball[:, 5] = f32c(bkr)
        ball[:, 6:8] = bqce.reshape(2, 128).T
        ball[:, 8:10] = bkce.reshape(2, 128).T
        ball[:, 10:12] = f32c(bv[gsl]).reshape(2, 128).T
        in_maps.append(dict(
            xT=np.ascontiguousarray(x[b].T),
            wq=f32c(wq), wkv=f32c(wkv),
            wqr=f32c(wqr), wkr=f32c(wkr),
            wqce=wqce, wkce=wkce,
            wv=f32c(wv[:, gsl]),
            wo=f32c(wo[gsl, :]),
            ball=ball,
            cosf=cosf, sins=sins, dmask=dmask, ones=ones,
        ))

    from concourse.bass_utils import run_bass_kernel_spmd
    res = run_bass_kernel_spmd(nc, in_maps, core_ids=list(range(NCORES)), trace=trace)
    LAST_RESULT[0] = res

    y = np.empty((B, S, E), np.float32)
    for b in range(B):
        acc = res.results[4 * b]["y"].astype(np.float32).copy()
        for g in range(1, 4):
            acc += res.results[4 * b + g]["y"]
        y[b] = acc + np.asarray(bo, dtype=np.float32)[None, :]
    return y
